# revision 9
# baseline (speedup 1.0000x reference)
# Dense GAT layer (4 heads, dim 64) on Trainium2 via Bass/Tile.
#
# Math: h = x@W; e_ij = LeakyReLU(src_i + dst_j, 0.2); masked softmax over j
# with valid = adj & mask_i & mask_j; out = LN((alpha @ h) * mask_i).
#
# Key identities used on device:
#   exp(LeakyReLU(t)) = max(exp(t), exp(0.2 t))            (t = src_i + dst_j)
#   exp(src_i + dstm_j) = exp(src_i) * exp(dstm_j)          (rank-1 separable)
#   dstm_j = dst_j if mask_j else -1e9  -> exp() == 0 kills masked columns
#   adj mask applied as elementwise multiply with transposed 0/1 fp16 matrix
#   mask_i and 1/rowsum fold into a per-row scale after the alpha@h matmul
#   (rowsum comes free as a ones-column in the alpha@h matmul rhs).
#
# Layout: "e^T" orientation — j (softmax axis) on partitions, i on the free
# axis, so alpha@h needs no transposes and rowsum is a matmul column.
# Sharding: data-parallel, 2 graphs per core across 8 cores.

import os
import numpy as np

H, D = 4, 64
NEG = -1.0e9
EPS = 1e-5
NCORES = 8

_PROG_CACHE = {}


def _build_program(ng, n, in_dim, trivial_ln):
    import concourse.bacc as bacc
    import concourse.mybir as mybir
    import concourse.tile as tile
    from concourse.bass import ts

    f16 = mybir.dt.float16
    f32 = mybir.dt.float32
    AF = mybir.ActivationFunctionType
    OP = mybir.AluOpType
    AX = mybir.AxisListType

    HD = H * D
    NCH = n // 128          # node chunks
    KC = in_dim // 128      # contraction chunks for x@W
    NW = min(512, n)        # matmul moving-column chunk width
    NH = n // NW            # number of column chunks
    E = D + 1               # head block in hones (64 h cols + 1 ones col)

    nc = bacc.Bacc()

    x16 = nc.dram_tensor("x16", [ng, n, in_dim], f16, kind="ExternalInput")
    adjm = nc.dram_tensor("adjm", [ng, n, n], f16, kind="ExternalInput")
    wc = nc.dram_tensor("wc", [128, KC * (HD + H)], f16, kind="ExternalInput")
    wsd = nc.dram_tensor("wsd", [128, KC * H], f16, kind="ExternalInput")
    ones16 = nc.dram_tensor("ones16", [1, 128], f16, kind="ExternalInput")
    mcolT = nc.dram_tensor("mcolT", [ng, 128, NCH], f32, kind="ExternalInput")
    negbT = nc.dram_tensor("negbT", [ng, 128, NCH * H], f32, kind="ExternalInput")
    if not trivial_ln:
        gam = nc.dram_tensor("gamma_rep", [128, HD], f32, kind="ExternalInput")
        bet = nc.dram_tensor("beta_rep", [128, HD], f32, kind="ExternalInput")
    out = nc.dram_tensor("out", [ng, n, HD], f32, kind="ExternalOutput")

    with tile.TileContext(nc) as tc:
        with (
            # SBUF pools
            tc.tile_pool(name="consts", bufs=1) as consts,
            tc.tile_pool(name="xt", bufs=2 * KC) as xt_pool,
            tc.tile_pool(name="adjt", bufs=NCH + 3) as adjt_pool,
            tc.tile_pool(name="rows", bufs=2) as rows_pool,
            tc.tile_pool(name="reps", bufs=3) as reps_pool,
            tc.tile_pool(name="hones", bufs=NCH + 2) as hones_pool,
            tc.tile_pool(name="small", bufs=NCH + 2) as small_pool,
            tc.tile_pool(name="ew", bufs=3) as ew_pool,
            tc.tile_pool(name="u", bufs=NCH + 4) as u_pool,
            tc.tile_pool(name="osb", bufs=NCH + 2) as osb_pool,
            tc.tile_pool(name="ln", bufs=4) as ln_pool,
            tc.tile_pool(name="misc", bufs=2) as misc_pool,
            # PSUM pools (8 banks total: 2 + 2 + 2 + 2)
            tc.tile_pool(name="ph", bufs=2, space="PSUM") as ph_pool,
            tc.tile_pool(name="psd", bufs=1, space="PSUM") as psd_pool,
            tc.tile_pool(name="prep", bufs=1, space="PSUM") as prep_pool,
            tc.tile_pool(name="pav", bufs=2, space="PSUM") as pav_pool,
        ):
            # ---- constants ----
            ones_sb = consts.tile([1, 128], f16, tag="ones")
            nc.sync.dma_start(ones_sb[:], ones16[:])
            wc_sb = consts.tile([128, KC * (HD + H)], f16, tag="wc")
            nc.sync.dma_start(wc_sb[:], wc[:])
            wsd_sb = consts.tile([128, KC * H], f16, tag="wsd")
            nc.sync.dma_start(wsd_sb[:], wsd[:])
            if not trivial_ln:
                gam_sb = consts.tile([128, HD], f32, tag="gam")
                nc.sync.dma_start(gam_sb[:], gam[:])
                bet_sb = consts.tile([128, HD], f32, tag="bet")
                nc.sync.dma_start(bet_sb[:], bet[:])
            eps_sb = consts.tile([128, 1], f32, tag="eps")
            nc.vector.memset(eps_sb[:], EPS)

            for g in range(ng):
                # ---- input DMAs ----
                # xT[kc]: [128, n] fp16, via DMA-transpose from x16[g]
                xt = []
                for kc in range(KC):
                    t = xt_pool.tile([128, n], f16, tag="xt")
                    nc.sync.dma_start(
                        t[:], x16[g, :, ts(kc, 128)], transpose=True
                    )
                    xt.append(t)
                # adjT[jc]: [128, n] fp16 (adjT[j, i] = adj[i, j])
                adjt = []
                for jc in range(NCH):
                    t = adjt_pool.tile([128, n], f16, tag="adjt")
                    nc.sync.dma_start(
                        t[:], adjm[g, :, ts(jc, 128)], transpose=True
                    )
                    adjt.append(t)
                mcol_sb = small_pool.tile([128, NCH], f32, tag="mcol")
                nc.sync.dma_start(mcol_sb[:], mcolT[g])
                negb_sb = small_pool.tile([128, NCH * H], f32, tag="negb")
                nc.sync.dma_start(negb_sb[:], negbT[g])

                # ---- src rows: psum_sd[h, i] = (x @ Wa_src)^T ----
                psd = psd_pool.tile([H, n], f32, tag="psd")
                for nh in range(NH):
                    for kc in range(KC):
                        nc.tensor.matmul(
                            psd[:, ts(nh, NW)],
                            wsd_sb[:, ts(kc, H)],
                            xt[kc][:, ts(nh, NW)],
                            start=(kc == 0),
                            stop=(kc == KC - 1),
                        )
                arow = rows_pool.tile([H, n], f16, tag="arow")
                nc.scalar.activation(arow[:], psd[:], AF.Exp)
                crow = rows_pool.tile([H, n], f16, tag="crow")
                nc.scalar.activation(crow[:], psd[:], AF.Exp, scale=0.2)
                # flatten head rows into partition 0 (PE rhs needs base part 0)
                arowx = rows_pool.tile([1, H * n], f16, tag="arowx")
                nc.sync.dma_start(arowx[:].rearrange("p (h w) -> p h w", h=H), arow[:])
                crowx = rows_pool.tile([1, H * n], f16, tag="crowx")
                nc.sync.dma_start(crowx[:].rearrange("p (h w) -> p h w", h=H), crow[:])

                # ---- h_ext per chunk: h (fp16, with ones col) + dstm/Bm/Dm ----
                hones = []
                Bm = []
                Dm = []
                for ic in range(NCH):
                    ph = ph_pool.tile([128, HD + H], f32, tag="ph")
                    for kc in range(KC):
                        nc.tensor.matmul(
                            ph[:],
                            xt[kc][:, ts(ic, 128)],
                            wc_sb[:, ts(kc, HD + H)],
                            start=(kc == 0),
                            stop=(kc == KC - 1),
                        )
                    ho = hones_pool.tile([128, H * E], f16, tag="hones")
                    ho3 = ho[:].rearrange("p (h e) -> p h e", h=H)
                    nc.vector.tensor_copy(
                        ho3[:, :, 0:D],
                        ph[:, 0:HD].rearrange("p (h d) -> p h d", h=H),
                    )
                    nc.vector.memset(ho3[:, :, D : D + 1], 1.0)
                    hones.append(ho)
                    # dstm = dst * m + (-1e9 * (1 - m))
                    dstm = small_pool.tile([128, H], f32, tag="dstm")
                    nc.vector.scalar_tensor_tensor(
                        dstm[:],
                        ph[:, HD : HD + H],
                        mcol_sb[:, ic : ic + 1],
                        negb_sb[:, ts(ic, H)],
                        op0=OP.mult,
                        op1=OP.add,
                    )
                    bm = small_pool.tile([128, H], f32, tag="bm")
                    nc.scalar.activation(bm[:], dstm[:], AF.Exp)
                    Bm.append(bm)
                    dm = small_pool.tile([128, H], f32, tag="dm")
                    nc.scalar.activation(dm[:], dstm[:], AF.Exp, scale=0.2)
                    Dm.append(dm)

                # ---- per head: replicate rows, elementwise, alpha@h ----
                o_sb = [
                    osb_pool.tile([128, HD], f32, tag="osb", name=f"osb_{g}_{i}")
                    for i in range(NCH)
                ]
                for h in range(H):
                    # A_rep / C_rep: [128, n] fp16 replicated exp(src) rows
                    pr = prep_pool.tile([128, n], f32, tag="prep")
                    for nh in range(NH):
                        nc.tensor.matmul(
                            pr[:, ts(nh, NW)],
                            ones_sb[:],
                            arowx[0:1, h * n + nh * NW : h * n + (nh + 1) * NW],
                            start=True,
                            stop=True,
                        )
                    arep = reps_pool.tile([128, n], f16, tag="arep")
                    nc.scalar.copy(arep[:], pr[:])
                    pr2 = prep_pool.tile([128, n], f32, tag="prep")
                    for nh in range(NH):
                        nc.tensor.matmul(
                            pr2[:, ts(nh, NW)],
                            ones_sb[:],
                            crowx[0:1, h * n + nh * NW : h * n + (nh + 1) * NW],
                            start=True,
                            stop=True,
                        )
                    crep = reps_pool.tile([128, n], f16, tag="crep")
                    nc.vector.tensor_copy(crep[:], pr2[:])

                    u_tiles = []
                    for jc in range(NCH):
                        t2 = ew_pool.tile([128, n], f16, tag="t2")
                        nc.vector.tensor_scalar(
                            t2[:],
                            crep[:],
                            Dm[jc][:, h : h + 1],
                            None,
                            op0=OP.mult,
                        )
                        w = ew_pool.tile([128, n], f16, tag="w")
                        nc.vector.scalar_tensor_tensor(
                            w[:],
                            arep[:],
                            Bm[jc][:, h : h + 1],
                            t2[:],
                            op0=OP.mult,
                            op1=OP.max,
                        )
                        u = u_pool.tile([128, n], f16, tag="u")
                        nc.vector.tensor_mul(u[:], w[:], adjt[jc][:])
                        u_tiles.append(u)

                    for ic in range(NCH):
                        pav = pav_pool.tile([128, E], f32, tag="pav")
                        for jc in range(NCH):
                            nc.tensor.matmul(
                                pav[:],
                                u_tiles[jc][:, ts(ic, 128)],
                                hones[jc][:, ts(h, E)],
                                start=(jc == 0),
                                stop=(jc == NCH - 1),
                            )
                        rs = ln_pool.tile([128, 1], f32, tag="rs")
                        nc.vector.reciprocal(rs[:], pav[:, D : D + 1])
                        rm = ln_pool.tile([128, 1], f32, tag="rm")
                        nc.vector.tensor_mul(
                            rm[:], rs[:], mcol_sb[:, ic : ic + 1]
                        )
                        nc.vector.tensor_scalar(
                            o_sb[ic][:, ts(h, D)],
                            pav[:, 0:D],
                            rm[:],
                            None,
                            op0=OP.mult,
                        )

                # ---- LayerNorm + output ----
                for ic in range(NCH):
                    s1 = ln_pool.tile([128, 1], f32, tag="s1")
                    nc.vector.tensor_reduce(s1[:], o_sb[ic][:], AX.X, OP.add)
                    sq = misc_pool.tile([128, HD], f32, tag="sq")
                    s2 = ln_pool.tile([128, 1], f32, tag="s2")
                    nc.scalar.activation(
                        sq[:], o_sb[ic][:], AF.Square, accum_out=s2[:]
                    )
                    mu = ln_pool.tile([128, 1], f32, tag="mu")
                    nc.vector.tensor_scalar(
                        mu[:], s1[:], 1.0 / HD, None, op0=OP.mult
                    )
                    mu2 = ln_pool.tile([128, 1], f32, tag="mu2")
                    nc.vector.tensor_mul(mu2[:], mu[:], mu[:])
                    var = ln_pool.tile([128, 1], f32, tag="var")
                    nc.vector.scalar_tensor_tensor(
                        var[:],
                        s2[:],
                        1.0 / HD,
                        mu2[:],
                        op0=OP.mult,
                        op1=OP.subtract,
                    )
                    sd = ln_pool.tile([128, 1], f32, tag="sd")
                    nc.scalar.activation(sd[:], var[:], AF.Sqrt, bias=eps_sb[:])
                    rstd = ln_pool.tile([128, 1], f32, tag="rstd")
                    nc.vector.reciprocal(rstd[:], sd[:])
                    o2 = misc_pool.tile([128, HD], f32, tag="o2")
                    nc.vector.tensor_scalar(
                        o2[:],
                        o_sb[ic][:],
                        mu[:],
                        rstd[:],
                        op0=OP.subtract,
                        op1=OP.mult,
                    )
                    if not trivial_ln:
                        nc.vector.tensor_mul(o2[:], o2[:], gam_sb[:])
                        nc.vector.tensor_add(o2[:], o2[:], bet_sb[:])
                    nc.sync.dma_start(out[g, ts(ic, 128), :], o2[:])

    nc.compile()
    return nc


def _host_prep(x, adj, mask, W, a_src, a_dst, gamma, beta, ng, trivial_ln):
    """Build per-core input maps (host-side folding + dtype packing only)."""
    b, n, in_dim = x.shape
    HD = H * D
    NCH = n // 128
    KC = in_dim // 128

    # Fold attention vectors into W:  Wa[c, h] = sum_d W[c, h*D+d] * a[h, d]
    Wr = W.astype(np.float64).reshape(in_dim, H, D)
    wa_src = np.einsum("chd,hd->ch", Wr, a_src.astype(np.float64))
    wa_dst = np.einsum("chd,hd->ch", Wr, a_dst.astype(np.float64))

    wc_full = np.ascontiguousarray(
        np.concatenate(
            [W.astype(np.float16), wa_dst.astype(np.float16)], axis=1
        )
        .reshape(KC, 128, HD + H)
        .transpose(1, 0, 2)
    ).reshape(128, KC * (HD + H))
    wsd_full = np.ascontiguousarray(
        wa_src.astype(np.float16).reshape(KC, 128, H).transpose(1, 0, 2)
    ).reshape(128, KC * H)
    ones16 = np.ones((1, 128), np.float16)

    mask_f = (mask > 0).astype(np.float32)  # [b, n]

    in_maps = []
    for c in range(NCORES):
        gs = slice(c * ng, (c + 1) * ng)
        mg = mask_f[gs]  # [ng, n]
        mcolT = np.ascontiguousarray(
            mg.reshape(ng, NCH, 128).transpose(0, 2, 1)
        )  # [ng, 128, NCH]
        negb = (NEG * (1.0 - mg)).reshape(ng, NCH, 128, 1)
        negbT = np.ascontiguousarray(
            np.broadcast_to(negb, (ng, NCH, 128, H)).transpose(0, 2, 1, 3)
        ).reshape(ng, 128, NCH * H)
        m = {
            "x16": x[gs].astype(np.float16),
            "adjm": (adj[gs] != 0).astype(np.float16),
            "wc": wc_full,
            "wsd": wsd_full,
            "ones16": ones16,
            "mcolT": mcolT.astype(np.float32),
            "negbT": negbT.astype(np.float32),
        }
        if not trivial_ln:
            m["gamma_rep"] = np.ascontiguousarray(
                np.broadcast_to(gamma.astype(np.float32), (128, HD))
            )
            m["beta_rep"] = np.ascontiguousarray(
                np.broadcast_to(beta.astype(np.float32), (128, HD))
            )
        in_maps.append(m)
    return in_maps


def kernel(x, adj, mask, W, a_src, a_dst, gamma, beta, _trace=False):
    from concourse.bass_utils import run_bass_kernel_spmd

    b, n, in_dim = x.shape
    ng = b // NCORES
    trivial_ln = bool(np.all(gamma == 1.0) and np.all(beta == 0.0))

    key = (ng, n, in_dim, trivial_ln)
    if key not in _PROG_CACHE:
        _PROG_CACHE[key] = _build_program(*key)
    nc = _PROG_CACHE[key]

    in_maps = _host_prep(
        x, adj, mask, W, a_src, a_dst, gamma, beta, ng, trivial_ln
    )
    res = run_bass_kernel_spmd(
        nc, in_maps, core_ids=list(range(NCORES)), trace=_trace
    )
    outs = [res.results[c]["out"].reshape(ng, n, H * D) for c in range(NCORES)]
    full = np.concatenate(outs, axis=0).astype(np.float32)
    if _trace:
        return full, res
    return full
